# revision 2
# baseline (speedup 1.0000x reference)
"""Multi-head causal attention (B=2, S=2048, D=1024, H=16, HD=64) on 8 TRN2 cores.

Sharding: data + tensor parallel. Core c handles batch b = c // 4 and head
group g = c % 4 (4 heads = 256 of the 1024 hidden dims). Wq/Wk/Wv are split
column-wise, Wo row-wise; each core computes a partial [D, S] output (its
heads' contribution, transposed), and the host sums the 4 partials per batch.

On-device layout (per core): everything is computed "transposed" so the PE
contraction dim always sits on partitions:
  xT [D, S] -> Q2T/K2T [128 (2 heads x 64 dims), S] -> scoresT [k, q]
  -> exp -> PV with a ones-column appended to V (denominator lands on
  partition 64) -> normalize -> O^T [256, S] -> Wo^T partial [D, S].
All matmuls run as float32r (full PE rate at free-dim 512, ~1e-4 rel err).
"""

import sys

sys.path.insert(0, "/opt/trn_rl_repo")

import numpy as np
import ml_dtypes

import concourse.bass as bass
import concourse.tile as tile
from concourse import bacc, mybir
from concourse.bass_utils import run_bass_kernel_spmd

B, S, D, H, HD = 2, 2048, 1024, 16, 64
NCORES = 8
HPC = H // (NCORES // B)          # heads per core = 4
GD = HPC * HD                     # head-group width = 256
CH = 512                          # q-chunk (max fp32 moving free dim)
NCH = S // CH                     # 4 q-chunks
KT = S // 128                     # 16 k-tiles
ND = D // 128                     # 8 d-tiles
NEG = -30000.0                    # mask value; exp(NEG/8) == 0 in fp32

f32 = mybir.dt.float32
f32r = mybir.dt.float32r
bf16 = mybir.dt.bfloat16

_prog_cache = {}


def _ktiles_for_chunk(c, variant):
    if variant == "causal":
        return list(range(4 * c + 4))
    return list(range(KT))


def _build(variant):
    """variant: 'causal' (band masks resident), 'full' (no masking),
    'masked' (arbitrary mask streamed from DRAM)."""
    nc = bacc.Bacc("TRN2", target_bir_lowering=False, debug=False,
                   num_devices=NCORES)

    xt_ext = nc.declare_dram_parameter("xt", [D, S], f32r, isOutput=False)
    wq_ext = nc.declare_dram_parameter("wq4", [D, GD], f32r, isOutput=False)
    wk_ext = nc.declare_dram_parameter("wk4", [D, GD], f32r, isOutput=False)
    wv_ext = nc.declare_dram_parameter("wv4", [D, GD], f32r, isOutput=False)
    wo_ext = nc.declare_dram_parameter("wo4", [GD, D], f32r, isOutput=False)
    bq_ext = nc.declare_dram_parameter("bq4", [GD], f32, isOutput=False)
    bk_ext = nc.declare_dram_parameter("bk4", [GD], f32, isOutput=False)
    bv_ext = nc.declare_dram_parameter("bv4", [GD], f32, isOutput=False)
    bo_ext = nc.declare_dram_parameter("bo1", [D], f32, isOutput=False)
    if variant == "causal":
        mk_ext = nc.declare_dram_parameter("mk", [128, 4, CH], bf16,
                                           isOutput=False)
    elif variant == "masked":
        mk_ext = nc.declare_dram_parameter("mkf", [KT, NCH, 128, CH], bf16,
                                           isOutput=False)
    out_ext = nc.declare_dram_parameter("out", [D, S], f32, isOutput=True)

    Ident = mybir.ActivationFunctionType.Identity
    Exp = mybir.ActivationFunctionType.Exp

    with tile.TileContext(nc) as tc:
        with tc.tile_pool(name="consts", bufs=1) as consts, \
             tc.tile_pool(name="qk", bufs=2) as qk_pool, \
             tc.tile_pool(name="ptp", bufs=4) as pt_pool, \
             tc.tile_pool(name="scr", bufs=2) as sc_pool, \
             tc.tile_pool(name="outp", bufs=3) as outp, \
             tc.tile_pool(name="pp", bufs=2, space="PSUM") as pp, \
             tc.tile_pool(name="sp", bufs=2, space="PSUM") as sp, \
             tc.tile_pool(name="vp", bufs=2, space="PSUM") as vp, \
             tc.tile_pool(name="fp", bufs=2, space="PSUM") as fp:

            # ---- resident loads ----
            xt = consts.tile([128, ND, S], f32r)
            for d in range(ND):
                nc.sync.dma_start(out=xt[:, d, :],
                                  in_=xt_ext[d * 128:(d + 1) * 128, :])
            wq_sb = consts.tile([128, ND, GD], f32r)
            wk_sb = consts.tile([128, ND, GD], f32r)
            wv_sb = consts.tile([128, ND, GD], f32r)
            for w_sb, w_ext in ((wq_sb, wq_ext), (wk_sb, wk_ext),
                                (wv_sb, wv_ext)):
                for d in range(ND):
                    nc.sync.dma_start(out=w_sb[:, d, :],
                                      in_=w_ext[d * 128:(d + 1) * 128, :])
            wo_sb = consts.tile([128, 2, D], f32r)
            for t in range(2):
                nc.sync.dma_start(out=wo_sb[:, t, :],
                                  in_=wo_ext[t * 128:(t + 1) * 128, :])
            if variant == "causal":
                mk_sb = consts.tile([128, 4, CH], bf16)
                nc.sync.dma_start(out=mk_sb, in_=mk_ext[:, :, :])
            bq_sb = consts.tile([128, 2], f32)
            bk_sb = consts.tile([128, 2], f32)
            nc.sync.dma_start(out=bq_sb, in_=bq_ext.rearrange("(t p) -> p t", p=128))
            nc.sync.dma_start(out=bk_sb, in_=bk_ext.rearrange("(t p) -> p t", p=128))
            bo_sb = consts.tile([128, ND], f32)
            nc.sync.dma_start(out=bo_sb, in_=bo_ext.rearrange("(t p) -> p t", p=128))
            bv_row = consts.tile([1, GD], f32)
            nc.sync.dma_start(out=bv_row, in_=bv_ext[None, :])
            bvb = consts.tile([128, GD], f32)
            nc.gpsimd.partition_broadcast(bvb[:, :], bv_row[:, :])
            ones_f = consts.tile([128, 1], f32)
            nc.vector.memset(ones_f, 1.0)

            vau = consts.tile([128, HPC, KT, HD + 1], f32r)
            ot_sb = consts.tile([128, 2, S], f32r)

            # ---- V projection: vau[:, h, t, :64] = (x @ wv + bv) per head,
            #      col 64 = ones (PV denominator trick) ----
            for t in range(KT):
                v4 = pp.tile([128, CH], f32, tag="pp")
                for d in range(ND):
                    nc.tensor.matmul(v4[:, :GD],
                                     xt[:, d, t * 128:(t + 1) * 128],
                                     wv_sb[:, d, :],
                                     start=(d == 0), stop=(d == ND - 1))
                for h in range(HPC):
                    nc.vector.tensor_add(vau[:, h, t, 0:HD],
                                         v4[:, h * HD:(h + 1) * HD],
                                         bvb[:, h * HD:(h + 1) * HD])
                    nc.vector.tensor_copy(out=vau[:, h, t, HD:HD + 1],
                                          in_=ones_f)

            for p in range(2):          # head pair
                # ---- Q^T / K^T projections for this pair: [128, S] ----
                q2t = qk_pool.tile([128, S], f32r, tag="q2t")
                k2t = qk_pool.tile([128, S], f32r, tag="k2t")
                for w_sb, b_sb, dst in ((wq_sb, bq_sb, q2t),
                                        (wk_sb, bk_sb, k2t)):
                    for c in range(NCH):
                        pr = pp.tile([128, CH], f32, tag="pp")
                        for d in range(ND):
                            nc.tensor.matmul(
                                pr,
                                w_sb[:, d, p * 128:(p + 1) * 128],
                                xt[:, d, c * CH:(c + 1) * CH],
                                start=(d == 0), stop=(d == ND - 1))
                        nc.scalar.activation(
                            out=dst[:, c * CH:(c + 1) * CH], in_=pr,
                            func=Ident, bias=b_sb[:, p:p + 1], scale=1.0)

                # ---- attention for both heads of the pair ----
                for c in range(NCH):
                    for hp in range(2):
                        h = 2 * p + hp
                        lo, hi = hp * 64, hp * 64 + 64
                        pv = vp.tile([HD + 1, CH], f32, tag="pv")
                        kts = _ktiles_for_chunk(c, variant)
                        for i, t in enumerate(kts):
                            s_ps = sp.tile([128, CH], f32, tag="sc")
                            nc.tensor.matmul(
                                s_ps,
                                k2t[lo:hi, t * 128:(t + 1) * 128],
                                q2t[lo:hi, c * CH:(c + 1) * CH],
                                start=True, stop=True)
                            if variant == "causal":
                                if t >= 4 * c:
                                    nc.vector.tensor_add(
                                        s_ps, s_ps, mk_sb[:, t - 4 * c, :])
                            elif variant == "masked":
                                mt = pt_pool.tile([128, CH], bf16, tag="mkt")
                                nc.sync.dma_start(out=mt, in_=mk_ext[t, c])
                                nc.vector.tensor_add(s_ps, s_ps, mt)
                            ptl = pt_pool.tile([128, CH], f32r, tag="pt")
                            nc.scalar.activation(out=ptl, in_=s_ps, func=Exp,
                                                 scale=0.125)
                            nc.tensor.matmul(pv, vau[:, h, t, :], ptl,
                                             start=(i == 0),
                                             stop=(i == len(kts) - 1))
                        # normalize: row 64 of pv is the softmax denominator
                        rc = sc_pool.tile([HD + 1, CH], f32, tag="rc")
                        nc.vector.reciprocal(rc[HD:HD + 1, :],
                                             pv[HD:HD + 1, :])
                        stage = sc_pool.tile([1, CH], f32, tag="stage")
                        nc.sync.dma_start(out=stage, in_=rc[HD:HD + 1, :])
                        bc = sc_pool.tile([HD, CH], f32, tag="bc")
                        nc.gpsimd.partition_broadcast(bc[:, :], stage[:, :])
                        if hp == 0:
                            nc.vector.tensor_mul(
                                ot_sb[0:HD, p, c * CH:(c + 1) * CH],
                                pv[0:HD, :], bc[:, :])
                        else:
                            scr = sc_pool.tile([HD, CH], f32r, tag="scr1")
                            nc.vector.tensor_mul(scr, pv[0:HD, :], bc[:, :])
                            nc.sync.dma_start(
                                out=ot_sb[HD:128, p, c * CH:(c + 1) * CH],
                                in_=scr)

            # ---- output projection: partial out^T [D, S] = wo4^T @ O^T ----
            for c in range(NCH):
                for d in range(ND):
                    f_ps = fp.tile([128, CH], f32, tag="fp")
                    for t in range(2):
                        nc.tensor.matmul(
                            f_ps,
                            wo_sb[:, t, d * 128:(d + 1) * 128],
                            ot_sb[:, t, c * CH:(c + 1) * CH],
                            start=(t == 0), stop=(t == 1))
                    o_sb = outp.tile([128, CH], f32, tag="out")
                    nc.scalar.activation(out=o_sb, in_=f_ps, func=Ident,
                                         bias=bo_sb[:, d:d + 1], scale=1.0)
                    nc.sync.dma_start(
                        out=out_ext[d * 128:(d + 1) * 128,
                                    c * CH:(c + 1) * CH],
                        in_=o_sb)

    nc.compile()
    return nc


def _get_prog(variant):
    if variant not in _prog_cache:
        _prog_cache[variant] = _build(variant)
    return _prog_cache[variant]


def _classify_mask(mask):
    m = np.asarray(mask).reshape(S, S).astype(bool)
    tril = np.tril(np.ones((S, S), bool))
    if (m == tril).all():
        return "causal", None
    if m.all():
        return "full", None
    return "masked", m


def _band_masks():
    # band patterns in scoresT layout for the 4 k-tiles overlapping a q-chunk:
    # mk[kk, j, qq] = 0 if (128*j + kk) <= qq else NEG
    kk = np.arange(128)[:, None, None]
    j = np.arange(4)[None, :, None]
    qq = np.arange(CH)[None, None, :]
    return np.where(128 * j + kk <= qq, 0.0, NEG).astype(ml_dtypes.bfloat16)


def _full_masks(m):
    # mkf[t, c, kk, qq] = 0 if m[c*CH+qq, t*128+kk] else NEG  (scoresT layout)
    mt = np.where(m.T, 0.0, NEG).astype(ml_dtypes.bfloat16)  # [k, q]
    return np.ascontiguousarray(
        mt.reshape(KT, 128, NCH, CH).transpose(0, 2, 1, 3))


def kernel(x, mask, wq, bq, wk, bk, wv, bv, wo, bo):
    x = np.asarray(x, dtype=np.float32)
    wq = np.asarray(wq, dtype=np.float32)
    wk = np.asarray(wk, dtype=np.float32)
    wv = np.asarray(wv, dtype=np.float32)
    wo = np.asarray(wo, dtype=np.float32)
    bq = np.asarray(bq, dtype=np.float32)
    bk = np.asarray(bk, dtype=np.float32)
    bv = np.asarray(bv, dtype=np.float32)
    bo = np.asarray(bo, dtype=np.float32)

    variant, m = _classify_mask(mask)
    nc = _get_prog(variant)

    xt = [np.ascontiguousarray(x[b].T) for b in range(B)]
    if variant == "causal":
        mk = _band_masks()
    elif variant == "masked":
        mkf = _full_masks(m)

    in_maps = []
    for c in range(NCORES):
        b, g = c // (NCORES // B), c % (NCORES // B)
        gs = slice(g * GD, (g + 1) * GD)
        im = {
            "xt": xt[b],
            "wq4": np.ascontiguousarray(wq[:, gs]),
            "wk4": np.ascontiguousarray(wk[:, gs]),
            "wv4": np.ascontiguousarray(wv[:, gs]),
            "wo4": np.ascontiguousarray(wo[gs, :]),
            "bq4": np.ascontiguousarray(bq[gs]),
            "bk4": np.ascontiguousarray(bk[gs]),
            "bv4": np.ascontiguousarray(bv[gs]),
            "bo1": bo if g == 0 else np.zeros_like(bo),
        }
        if variant == "causal":
            im["mk"] = mk
        elif variant == "masked":
            im["mkf"] = mkf
        in_maps.append(im)

    res = run_bass_kernel_spmd(nc, in_maps, core_ids=list(range(NCORES)))
    out = np.zeros((B, S, D), dtype=np.float32)
    for c in range(NCORES):
        out[c // (NCORES // B)] += res.results[c]["out"].T
    return out


# revision 15
# speedup vs baseline: 1.3067x; 1.3067x over previous
"""Multi-head causal attention (B=2, S=2048, D=1024, H=16, HD=64) on 8 TRN2 cores.

Sharding: data + tensor parallel. Core c handles batch b = c // 4 and head
group g = c % 4 (4 heads = 256 of the 1024 hidden dims). Wq/Wk/Wv are split
column-wise, Wo row-wise; each core computes a partial [D, S] output (its
heads' contribution, transposed), and the host sums the 4 partials per batch.

On-device layout (per core): everything is computed "transposed" so the PE
contraction dim always sits on partitions:
  xT [D, S] -> Q2T/K2T [128 (2 heads x 64 dims), S] -> scoresT [k, q]
  -> exp -> PV with a ones-column appended to V (denominator lands on
  partition 64) -> normalize -> O^T [256, S] -> Wo^T partial [D, S].
All matmuls run as float32r (full PE rate at free-dim >=256, ~1e-4 rel err).

Causal handling: for a q-chunk of 512, k-tiles strictly below the diagonal
are computed full-width with exp batched over k-tile pairs ([128,1024] ACT
calls); the 4 k-tiles overlapping the diagonal are computed only on their
live column range [w:512] (w = 128 * tile-offset), with one resident
[128,128] triangle mask added to the diagonal block. Columns left of w are
never computed, masked, exp'd, or fed to PV.
"""

import sys

sys.path.insert(0, "/opt/trn_rl_repo")

import numpy as np
import ml_dtypes

import concourse.bass as bass
import concourse.tile as tile
from concourse import bacc, mybir
from concourse.bass_utils import run_bass_kernel_spmd

B, S, D, H, HD = 2, 2048, 1024, 16, 64
NCORES = 8
HPC = H // (NCORES // B)          # heads per core = 4
GD = HPC * HD                     # head-group width = 256
CH = 512                          # q-chunk (max fp32 moving free dim)
NCH = S // CH                     # 4 q-chunks
KT = S // 128                     # 16 k-tiles
ND = D // 128                     # 8 d-tiles
NEG = -30000.0                    # mask value; exp(NEG/8) == 0 in fp32

f32 = mybir.dt.float32
f32r = mybir.dt.float32r
bf16 = mybir.dt.bfloat16

_prog_cache = {}


def _build(variant):
    """variant: 'causal' (triangle mask resident, diagonal narrowing),
    'full' (no masking), 'masked' (arbitrary mask streamed from DRAM)."""
    nc = bacc.Bacc("TRN2", target_bir_lowering=False, debug=False,
                   num_devices=NCORES)

    xt_ext = nc.declare_dram_parameter("xt", [D, S], f32r, isOutput=False)
    wq_ext = nc.declare_dram_parameter("wq4", [D, GD], f32r, isOutput=False)
    wk_ext = nc.declare_dram_parameter("wk4", [D, GD], f32r, isOutput=False)
    wv_ext = nc.declare_dram_parameter("wv4", [D, GD], f32r, isOutput=False)
    wo_ext = nc.declare_dram_parameter("wo4", [GD, D], f32r, isOutput=False)
    bq_ext = nc.declare_dram_parameter("bq4", [GD], f32, isOutput=False)
    bk_ext = nc.declare_dram_parameter("bk4", [GD], f32, isOutput=False)
    bv_ext = nc.declare_dram_parameter("bv4", [GD], f32, isOutput=False)
    bo_ext = nc.declare_dram_parameter("bo1", [D], f32, isOutput=False)
    if variant == "causal":
        mk_ext = nc.declare_dram_parameter("tri", [128, 128], bf16,
                                           isOutput=False)
    elif variant == "masked":
        mk_ext = nc.declare_dram_parameter("mkf", [KT, NCH, 128, CH], bf16,
                                           isOutput=False)
    out_ext = nc.declare_dram_parameter("out", [D, S], f32, isOutput=True)

    Ident = mybir.ActivationFunctionType.Identity
    Exp = mybir.ActivationFunctionType.Exp

    with tile.TileContext(nc) as tc:
        with tc.tile_pool(name="consts", bufs=1) as consts, \
             tc.tile_pool(name="qk", bufs=2) as qk_pool, \
             tc.tile_pool(name="ptp", bufs=6) as pt_pool, \
             tc.tile_pool(name="scr", bufs=2) as sc_pool, \
             tc.tile_pool(name="outp", bufs=2) as outp, \
             tc.tile_pool(name="pp", bufs=2, space="PSUM") as pp, \
             tc.tile_pool(name="sp", bufs=3, space="PSUM") as sp, \
             tc.tile_pool(name="bcp", bufs=1, space="PSUM") as bcp, \
             tc.tile_pool(name="vp", bufs=2, space="PSUM") as vp:

            # ---- resident loads (spread across DMA queues) ----
            qeng = [nc.sync, nc.scalar]
            qi = [0]

            def ldma(out, in_):
                qeng[qi[0] % len(qeng)].dma_start(out=out, in_=in_)
                qi[0] += 1

            xt_r = xt_ext.rearrange("(t p) s -> p t s", p=128)
            wv_sb = consts.tile([128, ND, GD], f32r)
            xt = consts.tile([128, ND, S], f32r)
            nc.sync.dma_start(out=wv_sb,
                              in_=wv_ext.rearrange("(t p) g -> p t g", p=128))
            nc.scalar.dma_start(out=xt[:, :, 0:CH], in_=xt_r[:, :, 0:CH])
            wq_sb = consts.tile([128, ND, GD], f32r)
            wk_sb = consts.tile([128, ND, GD], f32r)
            nc.sync.dma_start(out=wq_sb,
                              in_=wq_ext.rearrange("(t p) g -> p t g", p=128))
            nc.scalar.dma_start(out=wk_sb,
                                in_=wk_ext.rearrange("(t p) g -> p t g", p=128))
            wo_sb = consts.tile([128, 2, D], f32r)
            nc.gpsimd.dma_start(out=wo_sb,
                                in_=wo_ext.rearrange("(t p) d -> p t d", p=128))
            if variant == "causal":
                tri_sb = consts.tile([128, 128], bf16)
                nc.sync.dma_start(out=tri_sb, in_=mk_ext[:, :])
            bq_sb = consts.tile([128, 2], f32)
            bk_sb = consts.tile([128, 2], f32)
            nc.sync.dma_start(out=bq_sb, in_=bq_ext.rearrange("(t p) -> p t", p=128))
            nc.sync.dma_start(out=bk_sb, in_=bk_ext.rearrange("(t p) -> p t", p=128))
            bo_sb = consts.tile([128, ND], f32)
            nc.sync.dma_start(out=bo_sb, in_=bo_ext.rearrange("(t p) -> p t", p=128))
            bv_row = consts.tile([1, GD], f32)
            nc.sync.dma_start(out=bv_row, in_=bv_ext[None, :])
            bvb = consts.tile([128, GD], f32)
            nc.gpsimd.partition_broadcast(bvb[:, :], bv_row[:, :])
            ones_c = consts.tile([128, KT, HPC, 1], f32)
            nc.vector.memset(ones_c, 1.0)
            actwarm = consts.tile([1, 1], f32)
            nc.scalar.activation(out=actwarm, in_=ones_c[0:1, 0, 0, :],
                                 func=Exp, scale=1.0)
            ones65f = consts.tile([HD + 1, HD], f32)
            nc.vector.memset(ones65f, 1.0)
            ones65 = consts.tile([HD + 1, HD], f32r)
            nc.vector.tensor_copy(out=ones65, in_=ones65f)

            vau = consts.tile([128, KT, HPC, HD + 1], f32r)
            ot_sb = consts.tile([128, 2, S], f32r)

            # ones-column of V_aug (PV denominator trick), single strided copy
            nc.vector.tensor_copy(out=vau[:, :, :, HD:HD + 1], in_=ones_c)

            # ---- stripe-major main loop: for each 512-col stripe of S:
            #      load xt stripe -> V s-tiles -> QK projections (both pairs)
            #      -> attention chunk c (all 4 heads) -> output projection ----
            q2ts, k2ts = [], []
            for p in range(2):
                q2t_p = qk_pool.tile([128, S], f32r, tag="q2t", name=f"q2t{p}")
                k2t_p = qk_pool.tile([128, S], f32r, tag="k2t", name=f"k2t{p}")
                q2ts.append(q2t_p)
                k2ts.append(k2t_p)

            for c in range(NCH):
                # xt stripe c (stripe 0 already loaded with the weights)
                if c > 0:
                    nc.sync.dma_start(out=xt[:, :, c * CH:(c + 1) * CH],
                                      in_=xt_r[:, :, c * CH:(c + 1) * CH])

                # V projection for s-tiles of this stripe
                for t in range(4 * c, 4 * c + 4):
                    v4 = pp.tile([128, CH], f32, tag="pp")
                    for d in range(ND):
                        nc.tensor.matmul(v4[:, :GD],
                                         xt[:, d, t * 128:(t + 1) * 128],
                                         wv_sb[:, d, :],
                                         start=(d == 0), stop=(d == ND - 1))
                    nc.vector.tensor_add(
                        vau[:, t, :, 0:HD],
                        v4[:, 0:GD].rearrange("p (h e) -> p h e", h=HPC),
                        bvb.rearrange("p (h e) -> p h e", h=HPC))

                # Q^T / K^T projections, chunk c, both pairs
                for p in range(2):
                    for w_sb, b_sb, dst in ((wq_sb, bq_sb, q2ts[p]),
                                            (wk_sb, bk_sb, k2ts[p])):
                        pr = pp.tile([128, CH], f32, tag="pp")
                        for d in range(ND):
                            nc.tensor.matmul(
                                pr,
                                w_sb[:, d, p * 128:(p + 1) * 128],
                                xt[:, d, c * CH:(c + 1) * CH],
                                start=(d == 0), stop=(d == ND - 1))
                        nc.scalar.activation(
                            out=dst[:, c * CH:(c + 1) * CH], in_=pr,
                            func=Ident, bias=b_sb[:, p:p + 1], scale=1.0)

                # attention chunk c, all 4 heads
                for p in range(2):
                    q2t, k2t = q2ts[p], k2ts[p]
                    for hp in range(2):
                        h = 2 * p + hp
                        lo, hi = hp * 64, hp * 64 + 64
                        qs = q2t[lo:hi, c * CH:(c + 1) * CH]
                        pv = vp.tile([HD + 1, CH], f32, tag="pv")
                        first = True

                        def do_pv(t, ptl_ap, w, last):
                            nonlocal first
                            nc.tensor.matmul(pv[:, w:CH],
                                             vau[:, t, h, :], ptl_ap,
                                             start=first, stop=last)
                            first = False

                        if variant == "causal":
                            nfull = 4 * c
                            for t in range(nfull):
                                s_ps = sp.tile([128, CH], f32, tag="sc")
                                nc.tensor.matmul(
                                    s_ps,
                                    k2t[lo:hi, t * 128:(t + 1) * 128],
                                    qs, start=True, stop=True)
                                ptl = pt_pool.tile([128, CH], f32r, tag="pt")
                                nc.scalar.activation(out=ptl, in_=s_ps,
                                                     func=Exp, scale=0.125)
                                do_pv(t, ptl, 0, False)
                            for j in range(4):      # diagonal band
                                t = 4 * c + j
                                w = 128 * j
                                s_ps = sp.tile([128, CH], f32, tag="sc")
                                nc.tensor.matmul(
                                    s_ps[:, w:CH],
                                    k2t[lo:hi, t * 128:(t + 1) * 128],
                                    q2t[lo:hi, c * CH + w:(c + 1) * CH],
                                    start=True, stop=True)
                                nc.vector.tensor_add(s_ps[:, w:w + 128],
                                                     s_ps[:, w:w + 128],
                                                     tri_sb)
                                ptl = pt_pool.tile([128, CH], f32r,
                                                   tag="pt")
                                nc.scalar.activation(out=ptl[:, w:CH],
                                                     in_=s_ps[:, w:CH],
                                                     func=Exp, scale=0.125)
                                do_pv(t, ptl[:, w:CH], w, j == 3)
                        else:
                            for t in range(KT):
                                s_ps = sp.tile([128, CH], f32, tag="sc")
                                nc.tensor.matmul(
                                    s_ps,
                                    k2t[lo:hi, t * 128:(t + 1) * 128],
                                    qs, start=True, stop=True)
                                if variant == "masked":
                                    mt = pt_pool.tile([128, CH], bf16,
                                                      tag="mkt")
                                    nc.sync.dma_start(
                                        out=mt, in_=mk_ext[t, c])
                                    nc.vector.tensor_add(s_ps, s_ps, mt)
                                ptl = pt_pool.tile([128, CH], f32r, tag="pt")
                                nc.scalar.activation(out=ptl, in_=s_ps,
                                                     func=Exp, scale=0.125)
                                do_pv(t, ptl, 0, t == KT - 1)

                        # normalize: row 64 of pv is the softmax denominator
                        pv_sb = sc_pool.tile([HD + 1, CH], f32, tag="pvs")
                        nc.scalar.activation(out=pv_sb, in_=pv, func=Ident,
                                             scale=1.0)
                        rc = sc_pool.tile([HD + 1, CH], f32r, tag="rc")
                        with nc.allow_low_precision("f32r recip: 1e-4 ok"):
                            nc.vector.reciprocal(rc[HD:HD + 1, :],
                                                 pv_sb[HD:HD + 1, :])
                        bc = bcp.tile([HD, CH], f32, tag="bc")
                        nc.tensor.matmul(bc[:, :], ones65[HD:HD + 1, :],
                                         rc[HD:HD + 1, :],
                                         start=True, stop=True)
                        if hp == 0:
                            nc.vector.tensor_mul(
                                ot_sb[0:HD, p, c * CH:(c + 1) * CH],
                                pv_sb[0:HD, :], bc[:, :])
                        else:
                            scr = sc_pool.tile([HD, CH], f32r, tag="scr1")
                            nc.vector.tensor_mul(scr, pv_sb[0:HD, :], bc[:, :])
                            nc.sync.dma_start(
                                out=ot_sb[HD:128, p, c * CH:(c + 1) * CH],
                                in_=scr)

                # output projection for chunk c
                for dh in range(2):
                  o_big = outp.tile([128, ND // 2, CH], f32, tag="out")
                  for d in range(dh * (ND // 2), (dh + 1) * (ND // 2)):
                    f_ps = sp.tile([128, CH], f32, tag="sc")
                    for t in range(2):
                        nc.tensor.matmul(
                            f_ps,
                            wo_sb[:, t, d * 128:(d + 1) * 128],
                            ot_sb[:, t, c * CH:(c + 1) * CH],
                            start=(t == 0), stop=(t == 1))
                    dd = d - dh * (ND // 2)
                    if d % 2 == 0:
                        nc.vector.tensor_scalar_add(out=o_big[:, dd, :],
                                                    in0=f_ps,
                                                    scalar1=bo_sb[:, d:d + 1])
                    else:
                        nc.scalar.activation(out=o_big[:, dd, :], in_=f_ps,
                                             func=Ident,
                                             bias=bo_sb[:, d:d + 1], scale=1.0)
                  nc.scalar.dma_start(
                      out=out_ext.rearrange("(t p) s -> p t s", p=128)[
                          :, dh * (ND // 2):(dh + 1) * (ND // 2),
                          c * CH:(c + 1) * CH],
                      in_=o_big)
    nc.compile()
    return nc


def _get_prog(variant):
    if variant not in _prog_cache:
        _prog_cache[variant] = _build(variant)
    return _prog_cache[variant]


def _classify_mask(mask):
    m = np.asarray(mask).reshape(S, S).astype(bool)
    tril = np.tril(np.ones((S, S), bool))
    if (m == tril).all():
        return "causal", None
    if m.all():
        return "full", None
    return "masked", m


def _tri_mask():
    # diagonal-block triangle in scoresT layout: 0 if kk <= qq else NEG
    kk = np.arange(128)[:, None]
    qq = np.arange(128)[None, :]
    return np.where(kk <= qq, 0.0, NEG).astype(ml_dtypes.bfloat16)


def _full_masks(m):
    # mkf[t, c, kk, qq] = 0 if m[c*CH+qq, t*128+kk] else NEG  (scoresT layout)
    mt = np.where(m.T, 0.0, NEG).astype(ml_dtypes.bfloat16)  # [k, q]
    return np.ascontiguousarray(
        mt.reshape(KT, 128, NCH, CH).transpose(0, 2, 1, 3))


def kernel(x, mask, wq, bq, wk, bk, wv, bv, wo, bo):
    x = np.asarray(x, dtype=np.float32)
    wq = np.asarray(wq, dtype=np.float32)
    wk = np.asarray(wk, dtype=np.float32)
    wv = np.asarray(wv, dtype=np.float32)
    wo = np.asarray(wo, dtype=np.float32)
    bq = np.asarray(bq, dtype=np.float32)
    bk = np.asarray(bk, dtype=np.float32)
    bv = np.asarray(bv, dtype=np.float32)
    bo = np.asarray(bo, dtype=np.float32)

    variant, m = _classify_mask(mask)
    nc = _get_prog(variant)

    xt = [np.ascontiguousarray(x[b].T) for b in range(B)]
    if variant == "masked":
        mkf = _full_masks(m)

    in_maps = []
    for c in range(NCORES):
        b, g = c // (NCORES // B), c % (NCORES // B)
        gs = slice(g * GD, (g + 1) * GD)
        im = {
            "xt": xt[b],
            "wq4": np.ascontiguousarray(wq[:, gs]),
            "wk4": np.ascontiguousarray(wk[:, gs]),
            "wv4": np.ascontiguousarray(wv[:, gs]),
            "wo4": np.ascontiguousarray(wo[gs, :]),
            "bq4": np.ascontiguousarray(bq[gs]),
            "bk4": np.ascontiguousarray(bk[gs]),
            "bv4": np.ascontiguousarray(bv[gs]),
            "bo1": bo if g == 0 else np.zeros_like(bo),
        }
        if variant == "causal":
            im["tri"] = _tri_mask()
        elif variant == "masked":
            im["mkf"] = mkf
        in_maps.append(im)

    res = run_bass_kernel_spmd(nc, in_maps, core_ids=list(range(NCORES)))
    out = np.zeros((B, S, D), dtype=np.float32)
    for c in range(NCORES):
        out[c // (NCORES // B)] += res.results[c]["out"].T
    return out
